# revision 1
# baseline (speedup 1.0000x reference)
"""GCN (2-layer, symmetric-norm message passing) on 8 Trainium2 NeuronCores.

Contract: kernel(**inputs) takes the FULL inputs (x [50000,4,300] f32,
edge_index [2,250000] i32, W1/b1/W2/b2) and returns the FULL output
[50000,300] f32.

Strategy (per sharding hint): shard destination nodes across the 8 cores
(6250 each), replicate the small weights, partition edges by destination so
scatter-adds are core-local, and AllGather the pre-scaled source features
between layers.  The scatter-add itself is computed on the PE array as a
sequence of 0/1-indicator matmuls over 128-edge chunks (edges sorted by
destination on the host), with the per-row gather done by indirect DMA.
"""

import math

import numpy as np

import concourse.bacc as bacc
import concourse.bass as bass
import concourse.tile as tile
from concourse import bass_utils, mybir
from concourse.bass import IndirectOffsetOnAxis
from concourse.masks import make_identity

F32 = mybir.dt.float32
BF16 = mybir.dt.bfloat16
I32 = mybir.dt.int32
P = 128

N_CORES = 8


def _cdiv(a, b):
    return (a + b - 1) // b


# ---------------------------------------------------------------- host prep


def prep_inputs(x, edge_index, W1, b1, W2, b2, n_cores=N_CORES):
    """Shard + preprocess the full inputs into per-core in_maps.

    Returns (in_maps, meta) where meta carries the dims needed to build the
    device program.
    """
    N, T, C = x.shape
    assert N % n_cores == 0
    NPC = N // n_cores
    NBLK = _cdiv(NPC, P)

    row = np.asarray(edge_index[0], dtype=np.int64)
    col = np.asarray(edge_index[1], dtype=np.int64)

    # symmetric sqrt-degree norm; degree on source (row), +1 for self loops
    deg = (np.bincount(row, minlength=N) + 1).astype(np.float32)
    dis = (deg.astype(np.float32) ** -0.5).astype(np.float32)

    core_of = col // NPC

    # first pass: per-core per-block edge counts -> global CPB
    per_core = []
    max_blk = 0
    for c in range(n_cores):
        m = core_of == c
        r = row[m]
        d = col[m] - c * NPC
        order = np.argsort(d, kind="stable")
        r = r[order]
        d = d[order]
        cnt = np.bincount(d // P, minlength=NBLK)
        max_blk = max(max_blk, int(cnt.max()) if len(cnt) else 0)
        per_core.append((r, d, cnt))
    CPB = max(1, _cdiv(max_blk, P))

    # replicated tensors
    CC = [(c0, min(P, C - c0)) for c0 in range(0, C, P)]
    KC = len(CC)
    import ml_dtypes

    w1c = np.zeros((KC, P, C), ml_dtypes.bfloat16)
    w2c = np.zeros((KC, P, C), ml_dtypes.bfloat16)
    for k, (c0, cs) in enumerate(CC):
        w1c[k, :cs, :] = (W1.T[c0 : c0 + cs, :] / np.float32(T)).astype(np.float32)
        w2c[k, :cs, :] = W2.T[c0 : c0 + cs, :].astype(np.float32)
    b1t = np.broadcast_to(np.asarray(b1, np.float32), (P, C)).copy()
    b2t = np.broadcast_to(np.asarray(b2, np.float32), (P, C)).copy()
    iota = np.broadcast_to(np.arange(P, dtype=np.float32), (P, P)).copy()

    in_maps = []
    for c in range(n_cores):
        r, d, cnt = per_core[c]
        starts = np.concatenate([[0], np.cumsum(cnt)])
        idxt = np.zeros((P, NBLK * CPB), np.int32)
        dlt = np.full((P, NBLK * CPB), -1.0, np.float32)
        for blk in range(NBLK):
            s, e = int(starts[blk]), int(starts[blk + 1])
            n = e - s
            pad = CPB * P
            rb = np.zeros(pad, np.int64)
            rb[:n] = r[s:e]
            db = np.full(pad, -1.0, np.float32)
            db[:n] = (d[s:e] - blk * P).astype(np.float32)
            idxt[:, blk * CPB : (blk + 1) * CPB] = (
                rb.reshape(CPB, P).T.astype(np.int32)
            )
            dlt[:, blk * CPB : (blk + 1) * CPB] = db.reshape(CPB, P).T

        dis_c = dis[c * NPC : (c + 1) * NPC]
        dist = np.zeros((P, NBLK), np.float32)
        dist.reshape(-1)[: 0] = 0  # noop, keep shape
        flat = np.zeros(NBLK * P, np.float32)
        flat[:NPC] = dis_c
        dist[:, :] = flat.reshape(NBLK, P).T

        in_maps.append(
            {
                "xs": np.ascontiguousarray(x[c * NPC : (c + 1) * NPC]).astype(
                    np.float32
                ),
                "w1c": w1c,
                "w2c": w2c,
                "b1t": b1t,
                "b2t": b2t,
                "iot": iota,
                "dist": dist,
                "idxt": idxt,
                "dlt": dlt,
            }
        )

    meta = dict(N=N, T=T, C=C, NPC=NPC, NBLK=NBLK, CPB=CPB, CC=CC, n_cores=n_cores)
    return in_maps, meta


# ------------------------------------------------------------- device build


def build_nc(meta):
    N = meta["N"]
    T = meta["T"]
    C = meta["C"]
    NPC = meta["NPC"]
    NBLK = meta["NBLK"]
    CPB = meta["CPB"]
    CC = meta["CC"]
    KC = len(CC)
    n_cores = meta["n_cores"]
    rg = [list(range(n_cores))]

    nc = bacc.Bacc(
        "TRN2", target_bir_lowering=False, debug=False, num_devices=n_cores
    )

    xs = nc.dram_tensor("xs", [NPC, T, C], F32, kind="ExternalInput")
    w1c = nc.dram_tensor("w1c", [KC, P, C], BF16, kind="ExternalInput")
    w2c = nc.dram_tensor("w2c", [KC, P, C], BF16, kind="ExternalInput")
    b1t = nc.dram_tensor("b1t", [P, C], F32, kind="ExternalInput")
    b2t = nc.dram_tensor("b2t", [P, C], F32, kind="ExternalInput")
    iot = nc.dram_tensor("iot", [P, P], F32, kind="ExternalInput")
    dist = nc.dram_tensor("dist", [P, NBLK], F32, kind="ExternalInput")
    idxt = nc.dram_tensor("idxt", [P, NBLK * CPB], I32, kind="ExternalInput")
    dlt = nc.dram_tensor("dlt", [P, NBLK * CPB], F32, kind="ExternalInput")
    out_ext = nc.dram_tensor("out", [NPC, C], F32, kind="ExternalOutput")

    ACT = mybir.ActivationFunctionType

    with tile.TileContext(nc) as tc:
        with (
            tc.tile_pool(name="dramp", bufs=1, space="DRAM") as dramp,
            tc.tile_pool(name="singles", bufs=1) as singles,
            tc.tile_pool(name="work", bufs=3) as wp,
            tc.tile_pool(name="msgs", bufs=12) as mp,
            tc.tile_pool(name="psA", bufs=1, space="PSUM") as psA,
            tc.tile_pool(name="psT", bufs=2, space="PSUM") as psT,
            tc.tile_pool(name="psB", bufs=3, space="PSUM") as psB,
            tc.tile_pool(name="psC", bufs=2, space="PSUM") as psC,
        ):
            agin1 = dramp.tile([NPC, C], BF16, name="agin1")
            hp1f = dramp.tile([N, C], BF16, addr_space="Shared", name="hp1f")
            agin2 = dramp.tile([NPC, C], BF16, name="agin2")
            hp2f = dramp.tile([N, C], BF16, addr_space="Shared", name="hp2f")

            # constants / tables in SBUF
            ident = singles.tile([P, P], BF16, name="ident")
            make_identity(nc, ident[:])
            w1sb = singles.tile([P, KC, C], BF16, name="w1sb")
            w2sb = singles.tile([P, KC, C], BF16, name="w2sb")
            for k in range(KC):
                nc.sync.dma_start(out=w1sb[:, k, :], in_=w1c[k])
                nc.sync.dma_start(out=w2sb[:, k, :], in_=w2c[k])
            b1sb = singles.tile([P, C], F32, name="b1sb")
            nc.sync.dma_start(out=b1sb[:], in_=b1t[:])
            b2sb = singles.tile([P, C], F32, name="b2sb")
            nc.sync.dma_start(out=b2sb[:], in_=b2t[:])
            iosb = singles.tile([P, P], F32, name="iosb")
            nc.sync.dma_start(out=iosb[:], in_=iot[:])
            dissb = singles.tile([P, NBLK], F32, name="dissb")
            nc.sync.dma_start(out=dissb[:], in_=dist[:])
            idxsb = singles.tile([P, NBLK * CPB], I32, name="idxsb")
            nc.sync.dma_start(out=idxsb[:], in_=idxt[:])
            dlsb = singles.tile([P, NBLK * CPB], F32, name="dlsb")
            nc.sync.dma_start(out=dlsb[:], in_=dlt[:])

            # resident self-term tiles: hps = dis * hp = dis^2 * h
            hps1 = singles.tile([P, NBLK, C], F32, name="hps1")
            hps2 = singles.tile([P, NBLK, C], F32, name="hps2")
            if NPC % P != 0:
                # zero once so partial-block tail rows stay zero
                nc.vector.memset(hps1[:], 0.0)
                nc.vector.memset(hps2[:], 0.0)
            def ag_full(agin, hpf):
                nc.gpsimd.collective_compute(
                    "AllGather",
                    mybir.AluOpType.bypass,
                    replica_groups=rg,
                    ins=[agin.opt()],
                    outs=[hpf.opt()],
                )

            # ---------------- stage A: h = mean_t(x) @ W1.T + b1, prescale
            for b in range(NBLK):
                Pb = min(P, NPC - b * P)
                dcol = dissb[:Pb, b : b + 1]
                xt = wp.tile([P, T, C], F32, tag="xt")
                nc.sync.dma_start(out=xt[:Pb], in_=xs[b * P : b * P + Pb])
                s0 = wp.tile([P, C], F32, tag="s0")
                s1 = wp.tile([P, C], F32, tag="s1")
                xm = wp.tile([P, C], BF16, tag="xm")
                nc.vector.tensor_add(out=s0[:Pb], in0=xt[:Pb, 0], in1=xt[:Pb, 1])
                nc.vector.tensor_add(out=s1[:Pb], in0=xt[:Pb, 2], in1=xt[:Pb, 3])
                nc.vector.tensor_add(out=xm[:Pb], in0=s0[:Pb], in1=s1[:Pb])
                hpp = psA.tile([P, C], F32, tag="hpp")
                for k, (c0, cs) in enumerate(CC):
                    ptr = psT.tile([P, P], BF16, tag="ptr")
                    nc.tensor.transpose(
                        out=ptr[:cs, :Pb],
                        in_=xm[:Pb, c0 : c0 + cs],
                        identity=ident[:Pb, :Pb],
                    )
                    xT = wp.tile([P, P], BF16, tag="xT")
                    nc.scalar.copy(out=xT[:cs, :Pb], in_=ptr[:cs, :Pb])
                    nc.tensor.matmul(
                        out=hpp[:Pb],
                        lhsT=xT[:cs, :Pb],
                        rhs=w1sb[:cs, k, :],
                        start=(k == 0),
                        stop=(k == KC - 1),
                    )
                th = wp.tile([P, C], F32, tag="th")
                nc.vector.tensor_add(out=th[:Pb], in0=hpp[:Pb], in1=b1sb[:Pb])
                hp_t = wp.tile([P, C], BF16, tag="hp")
                nc.scalar.activation(out=hp_t[:Pb], in_=th[:Pb], func=ACT.Copy, scale=dcol)
                nc.sync.dma_start(out=agin1[b * P : b * P + Pb], in_=hp_t[:Pb])
                nc.scalar.activation(
                    out=hps1[:Pb, b, :], in_=hp_t[:Pb], func=ACT.Copy, scale=dcol
                )
                if b == NBLK - 1:
                    ag_full(agin1, hp1f)


            # ------------- prop core: gather + indicator matmuls -> psum
            def prop_psum(b, src_full, pool):
                pp = pool.tile([P, C], F32, tag="pp")
                for ch in range(CPB):
                    j = b * CPB + ch
                    msg = mp.tile([P, C], BF16, tag="msg")
                    nc.gpsimd.indirect_dma_start(
                        out=msg[:],
                        out_offset=None,
                        in_=src_full[:],
                        in_offset=IndirectOffsetOnAxis(
                            ap=idxsb[:, j : j + 1], axis=0
                        ),
                    )
                    ind = wp.tile([P, P], BF16, tag="ind")
                    nc.vector.tensor_tensor(
                        out=ind[:],
                        in0=iosb[:],
                        in1=dlsb[:, j : j + 1].to_broadcast([P, P]),
                        op=mybir.AluOpType.is_equal,
                    )
                    nc.tensor.matmul(
                        out=pp[:],
                        lhsT=ind[:],
                        rhs=msg[:],
                        start=(ch == 0),
                        stop=(ch == CPB - 1),
                    )
                return pp

            # ---------------- layer 1 prop + layer 2 linear (fused per block)
            for b in range(NBLK):
                Pb = min(P, NPC - b * P)
                dcol = dissb[:, b : b + 1]
                pp = prop_psum(b, hp1f, psB)
                t1 = wp.tile([P, C], F32, tag="t1")
                nc.vector.scalar_tensor_tensor(
                    out=t1[:],
                    in0=pp[:],
                    scalar=dcol,
                    in1=hps1[:, b, :],
                    op0=mybir.AluOpType.mult,
                    op1=mybir.AluOpType.add,
                )
                h1 = wp.tile([P, C], BF16, tag="h1")
                nc.vector.scalar_tensor_tensor(
                    out=h1[:],
                    in0=t1[:],
                    scalar=0.01,
                    in1=t1[:],
                    op0=mybir.AluOpType.mult,
                    op1=mybir.AluOpType.max,
                )
                h2p = psC.tile([P, C], F32, tag="h2p")
                for k, (c0, cs) in enumerate(CC):
                    ptr2 = psT.tile([P, P], BF16, tag="ptr")
                    nc.tensor.transpose(
                        out=ptr2[:cs, :], in_=h1[:, c0 : c0 + cs], identity=ident[:]
                    )
                    hT = wp.tile([P, P], BF16, tag="hT")
                    nc.scalar.copy(out=hT[:cs, :], in_=ptr2[:cs, :])
                    nc.tensor.matmul(
                        out=h2p[:],
                        lhsT=hT[:cs, :],
                        rhs=w2sb[:cs, k, :],
                        start=(k == 0),
                        stop=(k == KC - 1),
                    )
                t2 = wp.tile([P, C], F32, tag="t2")
                nc.vector.tensor_add(out=t2[:], in0=h2p[:], in1=b2sb[:])
                hp2_t = wp.tile([P, C], BF16, tag="hp2")
                nc.scalar.activation(
                    out=hp2_t[:Pb], in_=t2[:Pb], func=ACT.Copy, scale=dissb[:Pb, b : b + 1]
                )
                nc.sync.dma_start(out=agin2[b * P : b * P + Pb], in_=hp2_t[:Pb])
                nc.scalar.activation(
                    out=hps2[:Pb, b, :],
                    in_=hp2_t[:Pb],
                    func=ACT.Copy,
                    scale=dissb[:Pb, b : b + 1],
                )
                if b == NBLK - 1:
                    ag_full(agin2, hp2f)


            # ---------------- layer 2 prop -> output
            for b in range(NBLK):
                Pb = min(P, NPC - b * P)
                dcol = dissb[:, b : b + 1]
                pp = prop_psum(b, hp2f, psB)
                ot = wp.tile([P, C], F32, tag="ot")
                nc.vector.scalar_tensor_tensor(
                    out=ot[:],
                    in0=pp[:],
                    scalar=dcol,
                    in1=hps2[:, b, :],
                    op0=mybir.AluOpType.mult,
                    op1=mybir.AluOpType.add,
                )
                nc.sync.dma_start(out=out_ext[b * P : b * P + Pb], in_=ot[:Pb])

    nc.compile()
    return nc


# ------------------------------------------------------------------ runner

_CACHE = {}


def run(x, edge_index, W1, b1, W2, b2, n_cores=N_CORES, trace=False):
    in_maps, meta = prep_inputs(x, edge_index, W1, b1, W2, b2, n_cores)
    key = (meta["N"], meta["T"], meta["C"], meta["CPB"], n_cores)
    if key not in _CACHE:
        _CACHE[key] = build_nc(meta)
    nc = _CACHE[key]
    res = bass_utils.run_bass_kernel_spmd(
        nc, in_maps, core_ids=list(range(n_cores)), trace=trace
    )
    NPC = meta["NPC"]
    outs = [np.asarray(res.results[c]["out"]) for c in range(n_cores)]
    full = np.concatenate(outs, axis=0).astype(np.float32)
    return full, res


def kernel(x, edge_index, W1, b1, W2, b2):
    x = np.asarray(x)
    edge_index = np.asarray(edge_index)
    full, _ = run(
        np.asarray(x, np.float32),
        edge_index,
        np.asarray(W1, np.float32),
        np.asarray(b1, np.float32),
        np.asarray(W2, np.float32),
        np.asarray(b2, np.float32),
    )
    return full



# revision 4
# speedup vs baseline: 1.0447x; 1.0447x over previous
"""GCN (2-layer, symmetric-norm message passing) on 8 Trainium2 NeuronCores.

Contract: kernel(**inputs) takes the FULL inputs (x [50000,4,300] f32,
edge_index [2,250000] i32, W1/b1/W2/b2) and returns the FULL output
[50000,300] f32.

Strategy (per sharding hint): shard destination nodes across the 8 cores
(6250 each), replicate the small weights, partition edges by destination so
scatter-adds are core-local, and AllGather the pre-scaled source features
between layers.  The scatter-add is computed on the PE array as indicator
matmuls over 128-edge chunks (edges sorted by destination on the host).

v4 optimizations over the first working version:
  - Per-edge source rows are fetched with gpsimd.dma_gather (one batched
    SWDGE instruction per destination block per source half, ~1.1us of GpSimd
    time for 384 rows) instead of one indirect_dma_start per 128-edge chunk
    (~1.4us each): the 994ns fixed SWDGE descriptor-generation overhead was
    serializing both propagation phases.  dma_gather needs int16 indices and
    256B-aligned rows, so the gathered feature tables are padded to 384 bf16
    columns.
  - The gathered table is split into two Shared tensors by source
    row-within-shard (rows < RSPLIT vs >= RSPLIT of every core's shard), each
    filled by its own AllGather issued as soon as the producing half of the
    block loop finishes - overlapping most of the collective with stage-A /
    layer-1 compute.  (A Shared DRAM tensor may only be written by a single
    instruction, hence one tensor per collective; the split also keeps each
    half's flat row space within int16 range for dma_gather.)
  - x is shipped pre-transposed per block ([C, T*128] per block) so stage A
    needs no PE-array transposes: the T-mean is a few vector adds and the
    x@W1.T matmul consumes the summed tile directly as lhsT.
  - hps1/hps2 self-term caches in bf16 (SBUF pressure).
"""

import math

import numpy as np

import concourse.bacc as bacc
import concourse.bass as bass
import concourse.tile as tile
from concourse import bass_utils, library_config, mybir
from concourse.masks import make_identity

F32 = mybir.dt.float32
BF16 = mybir.dt.bfloat16
I16 = mybir.dt.int16
P = 128
EP = 384  # padded feature row width (bf16) -> 768B, multiple of 256B

N_CORES = 8


def _cdiv(a, b):
    return (a + b - 1) // b


# ---------------------------------------------------------------- host prep


def prep_inputs(x, edge_index, W1, b1, W2, b2, n_cores=N_CORES):
    """Shard + preprocess the full inputs into per-core in_maps."""
    N, T, C = x.shape
    assert N % n_cores == 0
    NPC = N // n_cores
    NBLK = _cdiv(NPC, P)
    SPLIT_BLK = _cdiv(NBLK, 2)  # blocks 0..SPLIT_BLK-1 -> half A
    RSPLIT = min(NPC, SPLIT_BLK * P)  # rows-within-shard boundary
    RB = NPC - RSPLIT  # rows per shard in half B
    assert n_cores * RSPLIT < 32768 and n_cores * RB < 32768  # int16 indices

    row = np.asarray(edge_index[0], dtype=np.int64)
    col = np.asarray(edge_index[1], dtype=np.int64)

    # symmetric sqrt-degree norm; degree on source (row), +1 for self loops
    deg = (np.bincount(row, minlength=N) + 1).astype(np.float32)
    dis = (deg.astype(np.float32) ** -0.5).astype(np.float32)

    core_of = col // NPC

    # per-core edge lists; source indices remapped into the flat row space of
    # their half's gathered tensor:
    #   half A (rr <  RSPLIT): c_src * RSPLIT + rr
    #   half B (rr >= RSPLIT): c_src * RB + (rr - RSPLIT)
    per_core = []
    maxA = maxB = 0
    for c in range(n_cores):
        m = core_of == c
        r = row[m]
        d = col[m] - c * NPC
        order = np.argsort(d, kind="stable")
        r = r[order]
        d = d[order]
        c_src = r // NPC
        rr = r % NPC
        in_a = rr < RSPLIT
        ridx = np.where(in_a, c_src * RSPLIT + rr, c_src * RB + (rr - RSPLIT))
        blk = d // P
        cntA = np.bincount(blk[in_a], minlength=NBLK)
        cntB = np.bincount(blk[~in_a], minlength=NBLK)
        maxA = max(maxA, int(cntA.max()))
        maxB = max(maxB, int(cntB.max()))
        per_core.append((ridx, d, in_a))
    CPA = max(1, _cdiv(maxA, P))
    CPB2 = max(1, _cdiv(maxB, P))
    CPBT = CPA + CPB2
    NICA = CPA * P // 16  # int16 idx columns per block (half A)
    NICB = CPB2 * P // 16

    # replicated tensors
    CC = [(c0, min(P, C - c0)) for c0 in range(0, C, P)]
    KC = len(CC)
    import ml_dtypes

    w1c = np.zeros((KC, P, C), ml_dtypes.bfloat16)
    w2c = np.zeros((KC, P, C), ml_dtypes.bfloat16)
    for k, (c0, cs) in enumerate(CC):
        w1c[k, :cs, :] = (W1.T[c0 : c0 + cs, :] / np.float32(T)).astype(np.float32)
        w2c[k, :cs, :] = W2.T[c0 : c0 + cs, :].astype(np.float32)
    b1t = np.broadcast_to(np.asarray(b1, np.float32), (P, C)).copy()
    b2t = np.broadcast_to(np.asarray(b2, np.float32), (P, C)).copy()
    iota = np.broadcast_to(np.arange(P, dtype=np.float32), (P, P)).copy()

    in_maps = []
    for c in range(n_cores):
        ridx, d, in_a = per_core[c]
        blk = d // P
        # slot i of a block-half <-> (chunk i//128, partition i%128);
        # idx tables are int16, [16, ncols] blocks replicated to 128 partitions
        idxga = np.zeros((P, NBLK * NICA), np.int16)
        idxgb = np.zeros((P, NBLK * NICB), np.int16)
        dlt = np.full((P, NBLK * CPBT), -1.0, np.float32)
        for b in range(NBLK):
            mb = blk == b
            for half, (mask, ncp, nic, idxg, coff) in enumerate(
                (
                    (mb & in_a, CPA, NICA, idxga, b * CPBT),
                    (mb & ~in_a, CPB2, NICB, idxgb, b * CPBT + CPA),
                )
            ):
                r_sub = ridx[mask]
                d_sub = d[mask]
                n = len(r_sub)
                pad = ncp * P
                rb_ = np.zeros(pad, np.int64)
                rb_[:n] = r_sub
                db = np.full(pad, -1.0, np.float32)
                db[:n] = (d_sub - b * P).astype(np.float32)
                blk16 = rb_.astype(np.int16).reshape(nic, 16).T  # [16, nic]
                idxg[:, b * nic : (b + 1) * nic] = np.tile(blk16, (P // 16, 1))
                dlt[:, coff : coff + ncp] = db.reshape(ncp, P).T

        dis_c = dis[c * NPC : (c + 1) * NPC]
        dist = np.zeros((P, NBLK), np.float32)
        flat = np.zeros(NBLK * P, np.float32)
        flat[:NPC] = dis_c
        dist[:, :] = flat.reshape(NBLK, P).T

        # x shard, pre-transposed per block: [NBLK, C, T*P] f32 with
        # xsT[b, ch, t*P + p] = x[c0 + b*P + p, t, ch], zero-padded tail rows.
        xs = np.zeros((NBLK * P, T, C), np.float32)
        xs[:NPC] = np.asarray(x[c * NPC : (c + 1) * NPC], np.float32)
        xsT = np.ascontiguousarray(
            xs.reshape(NBLK, P, T, C).transpose(0, 3, 2, 1).reshape(NBLK, C, T * P)
        )

        in_maps.append(
            {
                "xsT": xsT,
                "w1c": w1c,
                "w2c": w2c,
                "b1t": b1t,
                "b2t": b2t,
                "iot": iota,
                "dist": dist,
                "idxga": idxga,
                "idxgb": idxgb,
                "dlt": dlt,
            }
        )

    meta = dict(
        N=N, T=T, C=C, NPC=NPC, NBLK=NBLK, CPA=CPA, CPB2=CPB2, CC=CC,
        SPLIT_BLK=SPLIT_BLK, RSPLIT=RSPLIT, RB=RB, n_cores=n_cores,
    )
    return in_maps, meta


# ------------------------------------------------------------- device build


def build_nc(meta):
    N = meta["N"]
    T = meta["T"]
    C = meta["C"]
    NPC = meta["NPC"]
    NBLK = meta["NBLK"]
    CPA = meta["CPA"]
    CPB2 = meta["CPB2"]
    CPBT = CPA + CPB2
    CC = meta["CC"]
    KC = len(CC)
    SPLIT_BLK = meta["SPLIT_BLK"]
    RSPLIT = meta["RSPLIT"]
    RB = meta["RB"]
    n_cores = meta["n_cores"]
    NICA = CPA * P // 16
    NICB = CPB2 * P // 16
    rg = [list(range(n_cores))]

    nc = bacc.Bacc(
        "TRN2", target_bir_lowering=False, debug=False, num_devices=n_cores
    )

    xsT = nc.dram_tensor("xsT", [NBLK, C, T * P], F32, kind="ExternalInput")
    w1c = nc.dram_tensor("w1c", [KC, P, C], BF16, kind="ExternalInput")
    w2c = nc.dram_tensor("w2c", [KC, P, C], BF16, kind="ExternalInput")
    b1t = nc.dram_tensor("b1t", [P, C], F32, kind="ExternalInput")
    b2t = nc.dram_tensor("b2t", [P, C], F32, kind="ExternalInput")
    iot = nc.dram_tensor("iot", [P, P], F32, kind="ExternalInput")
    dist = nc.dram_tensor("dist", [P, NBLK], F32, kind="ExternalInput")
    idxga_d = nc.dram_tensor("idxga", [P, NBLK * NICA], I16, kind="ExternalInput")
    idxgb_d = nc.dram_tensor("idxgb", [P, NBLK * NICB], I16, kind="ExternalInput")
    dlt = nc.dram_tensor("dlt", [P, NBLK * CPBT], F32, kind="ExternalInput")
    out_ext = nc.dram_tensor("out", [NPC, C], F32, kind="ExternalOutput")

    ACT = mybir.ActivationFunctionType

    with tile.TileContext(nc) as tc:
        with (
            tc.tile_pool(name="dramp", bufs=1, space="DRAM") as dramp,
            tc.tile_pool(name="singles", bufs=1) as singles,
            tc.tile_pool(name="work", bufs=3) as wp,
            tc.tile_pool(name="msgs", bufs=4) as mp,
            tc.tile_pool(name="psA", bufs=1, space="PSUM") as psA,
            tc.tile_pool(name="psT", bufs=2, space="PSUM") as psT,
            tc.tile_pool(name="psB", bufs=3, space="PSUM") as psB,
            tc.tile_pool(name="psC", bufs=2, space="PSUM") as psC,
        ):
            # per-half staging + gathered tensors (one collective per Shared
            # tensor; separate in-tensors so each collective depends only on
            # its own half of the block loop)
            agin1a = dramp.tile([RSPLIT, EP], BF16, name="agin1a")
            agin1b = dramp.tile([RB, EP], BF16, name="agin1b")
            agin2a = dramp.tile([RSPLIT, EP], BF16, name="agin2a")
            agin2b = dramp.tile([RB, EP], BF16, name="agin2b")
            hp1fa = dramp.tile(
                [n_cores, RSPLIT, EP], BF16, addr_space="Shared", name="hp1fa"
            )
            hp1fb = dramp.tile(
                [n_cores, RB, EP], BF16, addr_space="Shared", name="hp1fb"
            )
            hp2fa = dramp.tile(
                [n_cores, RSPLIT, EP], BF16, addr_space="Shared", name="hp2fa"
            )
            hp2fb = dramp.tile(
                [n_cores, RB, EP], BF16, addr_space="Shared", name="hp2fb"
            )

            # constants / tables in SBUF
            ident = singles.tile([P, P], BF16, name="ident")
            make_identity(nc, ident[:])
            nc.gpsimd.load_library(library_config.mlp)
            w1sb = singles.tile([P, KC, C], BF16, name="w1sb")
            w2sb = singles.tile([P, KC, C], BF16, name="w2sb")
            for k in range(KC):
                nc.sync.dma_start(out=w1sb[:, k, :], in_=w1c[k])
                nc.sync.dma_start(out=w2sb[:, k, :], in_=w2c[k])
            b1sb = singles.tile([P, C], F32, name="b1sb")
            nc.sync.dma_start(out=b1sb[:], in_=b1t[:])
            b2sb = singles.tile([P, C], F32, name="b2sb")
            nc.sync.dma_start(out=b2sb[:], in_=b2t[:])
            iosb = singles.tile([P, P], F32, name="iosb")
            nc.sync.dma_start(out=iosb[:], in_=iot[:])
            dissb = singles.tile([P, NBLK], F32, name="dissb")
            nc.sync.dma_start(out=dissb[:], in_=dist[:])
            idxsa = singles.tile([P, NBLK * NICA], I16, name="idxsa")
            nc.sync.dma_start(out=idxsa[:], in_=idxga_d[:])
            idxsb2 = singles.tile([P, NBLK * NICB], I16, name="idxsb2")
            nc.sync.dma_start(out=idxsb2[:], in_=idxgb_d[:])
            dlsb = singles.tile([P, NBLK * CPBT], F32, name="dlsb")
            nc.sync.dma_start(out=dlsb[:], in_=dlt[:])

            # resident self-term tiles: hps = dis * hp = dis^2 * h
            hps1 = singles.tile([P, NBLK, C], BF16, name="hps1")
            hps2 = singles.tile([P, NBLK, C], BF16, name="hps2")
            if NPC % P != 0:
                # zero once so partial-block tail rows stay zero
                nc.vector.memset(hps1[:], 0.0)
                nc.vector.memset(hps2[:], 0.0)

            def ag(agin, hpf):
                nc.gpsimd.collective_compute(
                    "AllGather",
                    mybir.AluOpType.bypass,
                    replica_groups=rg,
                    ins=[agin[:]],
                    outs=[hpf[:]],
                )

            # ---------------- stage A: h = mean_t(x) @ W1.T + b1, prescale
            for b in range(NBLK):
                Pb = min(P, NPC - b * P)
                dcol = dissb[:Pb, b : b + 1]
                hpp = psA.tile([P, C], F32, tag="hpp")
                for k, (c0, cs) in enumerate(CC):
                    xt = wp.tile([P, T * P], F32, tag=f"xt{k}")
                    nc.sync.dma_start(out=xt[:cs], in_=xsT[b, c0 : c0 + cs, :])
                    s0 = wp.tile([P, P], F32, tag=f"s0_{k}")
                    s1 = wp.tile([P, P], F32, tag=f"s1_{k}")
                    xmT = wp.tile([P, P], BF16, tag=f"xmT{k}")
                    nc.vector.tensor_add(
                        out=s0[:cs], in0=xt[:cs, 0:P], in1=xt[:cs, P : 2 * P]
                    )
                    nc.vector.tensor_add(
                        out=s1[:cs], in0=xt[:cs, 2 * P : 3 * P], in1=xt[:cs, 3 * P :]
                    )
                    nc.vector.tensor_add(out=xmT[:cs], in0=s0[:cs], in1=s1[:cs])
                    nc.tensor.matmul(
                        out=hpp[:],
                        lhsT=xmT[:cs, :],
                        rhs=w1sb[:cs, k, :],
                        start=(k == 0),
                        stop=(k == KC - 1),
                    )
                th = wp.tile([P, C], F32, tag="th")
                nc.vector.tensor_add(out=th[:Pb], in0=hpp[:Pb], in1=b1sb[:Pb])
                hp_t = wp.tile([P, C], BF16, tag="hp")
                nc.scalar.activation(out=hp_t[:Pb], in_=th[:Pb], func=ACT.Copy, scale=dcol)
                if b < SPLIT_BLK:
                    nc.sync.dma_start(
                        out=agin1a[b * P : b * P + Pb, :C], in_=hp_t[:Pb]
                    )
                else:
                    r0 = b * P - RSPLIT
                    nc.sync.dma_start(out=agin1b[r0 : r0 + Pb, :C], in_=hp_t[:Pb])
                nc.scalar.activation(
                    out=hps1[:Pb, b, :], in_=hp_t[:Pb], func=ACT.Copy, scale=dcol
                )
                if b == SPLIT_BLK - 1:
                    ag(agin1a, hp1fa)
                if b == NBLK - 1:
                    ag(agin1b, hp1fb)

            hp1a_flat = hp1fa[:].flatten_outer_dims()
            hp1b_flat = hp1fb[:].flatten_outer_dims()
            hp2a_flat = hp2fa[:].flatten_outer_dims()
            hp2b_flat = hp2fb[:].flatten_outer_dims()

            # ------------- prop core: batched gathers + indicator matmuls
            def prop_psum(b, srcs, pool):
                msga = mp.tile([P, CPA, EP], BF16, tag="msga")
                nc.gpsimd.dma_gather(
                    out_ap=msga[:],
                    in_ap=srcs[0],
                    idxs_ap=idxsa[:, b * NICA : (b + 1) * NICA],
                    num_idxs=CPA * P,
                    num_idxs_reg=CPA * P,
                    elem_size=EP,
                )
                msgb = mp.tile([P, CPB2, EP], BF16, tag="msgb")
                nc.gpsimd.dma_gather(
                    out_ap=msgb[:],
                    in_ap=srcs[1],
                    idxs_ap=idxsb2[:, b * NICB : (b + 1) * NICB],
                    num_idxs=CPB2 * P,
                    num_idxs_reg=CPB2 * P,
                    elem_size=EP,
                )
                j0 = b * CPBT
                pp = pool.tile([P, C], F32, tag="pp")
                for ch in range(CPBT):
                    msg = (
                        msga[:, ch, :C] if ch < CPA else msgb[:, ch - CPA, :C]
                    )
                    ind = wp.tile([P, P], BF16, tag="ind")
                    nc.vector.tensor_tensor(
                        out=ind[:],
                        in0=iosb[:],
                        in1=dlsb[:, j0 + ch : j0 + ch + 1].to_broadcast([P, P]),
                        op=mybir.AluOpType.is_equal,
                    )
                    nc.tensor.matmul(
                        out=pp[:],
                        lhsT=ind[:],
                        rhs=msg,
                        start=(ch == 0),
                        stop=(ch == CPBT - 1),
                    )
                return pp

            # ---------------- layer 1 prop + layer 2 linear (fused per block)
            for b in range(NBLK):
                Pb = min(P, NPC - b * P)
                dcol = dissb[:, b : b + 1]
                pp = prop_psum(b, (hp1a_flat, hp1b_flat), psB)
                t1 = wp.tile([P, C], F32, tag="t1")
                nc.vector.scalar_tensor_tensor(
                    out=t1[:],
                    in0=pp[:],
                    scalar=dcol,
                    in1=hps1[:, b, :],
                    op0=mybir.AluOpType.mult,
                    op1=mybir.AluOpType.add,
                )
                h1 = wp.tile([P, C], BF16, tag="h1")
                nc.vector.scalar_tensor_tensor(
                    out=h1[:],
                    in0=t1[:],
                    scalar=0.01,
                    in1=t1[:],
                    op0=mybir.AluOpType.mult,
                    op1=mybir.AluOpType.max,
                )
                h2p = psC.tile([P, C], F32, tag="h2p")
                for k, (c0, cs) in enumerate(CC):
                    ptr2 = psT.tile([P, P], BF16, tag="ptr")
                    nc.tensor.transpose(
                        out=ptr2[:cs, :], in_=h1[:, c0 : c0 + cs], identity=ident[:]
                    )
                    hT = wp.tile([P, P], BF16, tag="hT")
                    nc.scalar.copy(out=hT[:cs, :], in_=ptr2[:cs, :])
                    nc.tensor.matmul(
                        out=h2p[:],
                        lhsT=hT[:cs, :],
                        rhs=w2sb[:cs, k, :],
                        start=(k == 0),
                        stop=(k == KC - 1),
                    )
                t2 = wp.tile([P, C], F32, tag="t2")
                nc.vector.tensor_add(out=t2[:], in0=h2p[:], in1=b2sb[:])
                hp2_t = wp.tile([P, C], BF16, tag="hp2")
                nc.scalar.activation(
                    out=hp2_t[:Pb], in_=t2[:Pb], func=ACT.Copy, scale=dissb[:Pb, b : b + 1]
                )
                if b < SPLIT_BLK:
                    nc.sync.dma_start(
                        out=agin2a[b * P : b * P + Pb, :C], in_=hp2_t[:Pb]
                    )
                else:
                    r0 = b * P - RSPLIT
                    nc.sync.dma_start(out=agin2b[r0 : r0 + Pb, :C], in_=hp2_t[:Pb])
                nc.scalar.activation(
                    out=hps2[:Pb, b, :],
                    in_=hp2_t[:Pb],
                    func=ACT.Copy,
                    scale=dissb[:Pb, b : b + 1],
                )
                if b == SPLIT_BLK - 1:
                    ag(agin2a, hp2fa)
                if b == NBLK - 1:
                    ag(agin2b, hp2fb)

            # ---------------- layer 2 prop -> output
            for b in range(NBLK):
                Pb = min(P, NPC - b * P)
                dcol = dissb[:, b : b + 1]
                pp = prop_psum(b, (hp2a_flat, hp2b_flat), psB)
                ot = wp.tile([P, C], F32, tag="ot")
                nc.vector.scalar_tensor_tensor(
                    out=ot[:],
                    in0=pp[:],
                    scalar=dcol,
                    in1=hps2[:, b, :],
                    op0=mybir.AluOpType.mult,
                    op1=mybir.AluOpType.add,
                )
                nc.sync.dma_start(out=out_ext[b * P : b * P + Pb], in_=ot[:Pb])

    nc.compile()
    return nc


# ------------------------------------------------------------------ runner

_CACHE = {}


def run(x, edge_index, W1, b1, W2, b2, n_cores=N_CORES, trace=False):
    in_maps, meta = prep_inputs(x, edge_index, W1, b1, W2, b2, n_cores)
    key = (meta["N"], meta["T"], meta["C"], meta["CPA"], meta["CPB2"], n_cores)
    if key not in _CACHE:
        _CACHE[key] = build_nc(meta)
    nc = _CACHE[key]
    res = bass_utils.run_bass_kernel_spmd(
        nc, in_maps, core_ids=list(range(n_cores)), trace=trace
    )
    outs = [np.asarray(res.results[c]["out"]) for c in range(n_cores)]
    full = np.concatenate(outs, axis=0).astype(np.float32)
    return full, res


def kernel(x, edge_index, W1, b1, W2, b2):
    x = np.asarray(x)
    edge_index = np.asarray(edge_index)
    full, _ = run(
        np.asarray(x, np.float32),
        edge_index,
        np.asarray(W1, np.float32),
        np.asarray(b1, np.float32),
        np.asarray(W2, np.float32),
        np.asarray(b2, np.float32),
    )
    return full
